# revision 27
# baseline (speedup 1.0000x reference)
"""ComplexAttentionV3 Trainium2 kernel (v3).

Sharding: 8 cores = data-parallel over batch (2) x tensor-parallel over
heads (16 -> 4 per core). Each core computes q/k/v for its 4 heads
(column-sharded projections), local attention, and a row-sharded
o-projection producing a partial [T, D] output; the host sums the 4
partials per batch.

v3 notes vs v2:
- Karatsuba complex projections for q/k/v: m1 = xr@wr, m2 = xi@wi,
  m3 = (xr+xi)@(wr+wi); qr = m1-m2, qi = m3-m1-m2. 3 contraction
  passes instead of 4 (-25% PE work). xs and ws are precomputed on the
  host. Combines + RoPE run on DVE and Pool (gpsimd) in alternating
  blocks to balance engine load.
- x streamed t-major (512-col superblocks, double-buffered) so three
  x images fit in SBUF alongside everything else.
- Attention jc-loop software-pipelined: s(jc+1) issues before
  av/dn(jc) so the PE keeps working while the scalar engine computes
  exp. PSUM: 2 score bufs (4 banks) + av (2) + dn (2) = 8.
- av PSUM bank freed via a fast Pool-engine copy before the (slower)
  normalize chain.
- ow weights DMA'd at kernel start (no attn->oproj DMA bubble); wq
  DMA'd per-dc so the first matmul starts within ~1 MB of traffic.
"""

import numpy as np
import ml_dtypes

import concourse.bacc as bacc
import concourse.tile as tile
from concourse import bass_isa, mybir
from concourse.bass import ts
from concourse.bass_utils import run_bass_kernel_spmd

B, T, D, H = 2, 2048, 1024, 16
HD = 64
NCORE = 8
TP = 4               # head-parallel degree (per batch)
HC = H // TP         # heads per core = 4
C = HC * HD          # local channels = 256
DC = D // 128        # contraction chunks = 8
TQ = T // 128        # 128-row t-chunks = 16
TB = T // 512        # 512-col t-superblocks = 4
TW = T // 1024       # 1024-col attention q-blocks = 2

F32 = mybir.dt.float32
BF16 = mybir.dt.bfloat16
EXP = mybir.ActivationFunctionType.Exp

LAST_RESULTS = None
_COMPILED = None


def _build():
    nc = bacc.Bacc("TRN2", target_bir_lowering=False, debug=False,
                   num_devices=NCORE)

    def din(name, shape, dt=BF16):
        return nc.dram_tensor(name, shape, dt, kind="ExternalInput").ap()

    xr_d = din("xrT", [128, DC, T])
    xi_d = din("xiT", [128, DC, T])
    wq = {k: din(f"wq_{k}", [128, DC, C]) for k in ("r", "i", "s")}
    wk = {k: din(f"wk_{k}", [128, DC, C]) for k in ("r", "i", "s")}
    wv = {k: din(f"wv_{k}", [128, DC, C]) for k in ("r", "i", "s")}
    ow = {k: din(f"ow_{k}", [128, 2, D]) for k in ("r", "i", "n")}
    cos_d = din("cos2", [128, T], BF16)
    sin_d = din("sin2", [128, T], BF16)
    outr_d = nc.dram_tensor("out_r", [T, D], F32, kind="ExternalOutput").ap()
    outi_d = nc.dram_tensor("out_i", [T, D], F32, kind="ExternalOutput").ap()

    with tile.TileContext(nc) as tc:
        with tc.tile_pool(name="persist", bufs=1) as persist:
            qkcat = persist.tile([128, 2 * HC, T], BF16, name="qkcat")
            vcat = persist.tile([128, TQ, HC, 128], BF16, name="vcat")
            urt = persist.tile([128, 2, T], BF16, name="urt")
            uit = persist.tile([128, 2, T], BF16, name="uit")
            ones = persist.tile([128, 1], BF16, name="ones")
            ows = {k: persist.tile([128, 2, D], BF16, name=f"ow{k}")
                   for k in ("r", "i", "n")}
            nc.vector.memset(ones[:], 1.0)

            # ---------------- projection phase ----------------
            with tc.tile_pool(name="xw", bufs=1) as xw, \
                 tc.tile_pool(name="xst", bufs=2) as xst, \
                 tc.tile_pool(name="rt", bufs=3) as rt, \
                 tc.tile_pool(name="pp", bufs=2, space="PSUM") as pp:
                wqs = {k: xw.tile([128, DC, C], BF16, name=f"wq{k}")
                       for k in ("r", "i", "s")}
                wks = {k: xw.tile([128, DC, C], BF16, name=f"wk{k}")
                       for k in ("r", "i", "s")}
                wvs = {k: xw.tile([128, DC, C], BF16, name=f"wv{k}")
                       for k in ("r", "i", "s")}
                cos = xw.tile([128, T], BF16, name="cos")
                sin = xw.tile([128, T], BF16, name="sin")
                # DMA order = need order. wq per-dc so the PE starts after
                # ~200 KB; tables arrive before the first combine; wk/wv/ow
                # whole-tensor behind them. x rides the sync queue.
                for k in ("r", "i", "s"):
                    for dc in range(DC):
                        nc.scalar.dma_start(wqs[k][:, dc, :], wq[k][:, dc, :])
                nc.scalar.dma_start(cos[:], cos_d[:])
                nc.scalar.dma_start(sin[:], sin_d[:])
                for k in ("r", "i", "s"):
                    nc.scalar.dma_start(wks[k][:], wk[k][:])

                for tb in range(TB):
                    tsl = ts(tb, 512)
                    # wv arrives just before tb0's v-section; ow (needed
                    # only at the o-projection) queues after tb0 so neither
                    # steals DMA bandwidth from the x/wq/wk startup window.
                    if tb == 0:
                        for k in ("r", "i", "s"):
                            nc.scalar.dma_start(wvs[k][:], wv[k][:])
                    if tb == 1:
                        for k in ("r", "i", "n"):
                            nc.scalar.dma_start(ows[k][:], ow[k][:])
                    xr = xst.tile([128, DC, 512], BF16, name="xr")
                    xi = xst.tile([128, DC, 512], BF16, name="xi")
                    xs = xst.tile([128, DC, 512], BF16, name="xs")
                    for dc in range(DC):
                        nc.sync.dma_start(xr[:, dc, :], xr_d[:, dc, tsl])
                        nc.sync.dma_start(xi[:, dc, :], xi_d[:, dc, tsl])
                    # xs = xr + xi on-device (bf16 2x DVE) instead of a third
                    # 4 MB DMA stream
                    nc.vector.tensor_add(xs[:], xr[:], xi[:])

                    # q/k: transposed [c, t] psums + Karatsuba + RoPE
                    for wsrc, hbase in ((wqs, 0), (wks, HC)):
                        for cc in range(2):
                            h0, h1 = hbase + 2 * cc, hbase + 2 * cc + 1
                            csl = ts(cc, 128)
                            m1 = pp.tile([128, 512], F32, name="m1")
                            m2 = pp.tile([128, 512], F32, name="m2")
                            m3 = pp.tile([128, 512], F32, name="m3")
                            for dc in range(DC):
                                nc.tensor.matmul(
                                    m1[:], lhsT=wsrc["r"][:, dc, csl],
                                    rhs=xr[:, dc, :],
                                    start=(dc == 0), stop=(dc == DC - 1))
                            for dc in range(DC):
                                nc.tensor.matmul(
                                    m2[:], lhsT=wsrc["i"][:, dc, csl],
                                    rhs=xi[:, dc, :],
                                    start=(dc == 0), stop=(dc == DC - 1))
                            for dc in range(DC):
                                nc.tensor.matmul(
                                    m3[:], lhsT=wsrc["s"][:, dc, csl],
                                    rhs=xs[:, dc, :],
                                    start=(dc == 0), stop=(dc == DC - 1))
                            # combine + rope, alternating DVE / Pool
                            # combine + rope (P = qr, Q = qi pre-rope).
                            # GPSIMD can't touch PSUM and DVE ops may read
                            # at most one PSUM operand, so: scalar engine
                            # stages m2 into SBUF; DVE does the 3
                            # PSUM-reading combines (P/Q written as bf16)
                            # and the 4 rope muls (all-bf16 -> 2x rate);
                            # Pool does the 4 SBUF-only half add/subs.
                            m2s = rt.tile([128, 512], F32, name="m2s")
                            nc.scalar.copy(m2s[:], m2[:])
                            Pb = rt.tile([128, 512], BF16, name="Pb")
                            Sx = rt.tile([128, 512], F32, name="Sx")
                            Qb = rt.tile([128, 512], BF16, name="Qb")
                            nc.vector.tensor_sub(Pb[:], m1[:], m2s[:])
                            nc.vector.tensor_add(Sx[:], m1[:], m2s[:])
                            nc.vector.tensor_sub(Qb[:], m3[:], Sx[:])
                            t1 = rt.tile([128, 512], BF16, name="t1")
                            t2 = rt.tile([128, 512], BF16, name="t2")
                            t3 = rt.tile([128, 512], BF16, name="t3")
                            t4 = rt.tile([128, 512], BF16, name="t4")
                            nc.vector.tensor_mul(t1[:], Pb[:], cos[:, tsl])
                            nc.vector.tensor_mul(t2[:], Qb[:], sin[:, tsl])
                            nc.vector.tensor_mul(t3[:], Pb[:], sin[:, tsl])
                            nc.vector.tensor_mul(t4[:], Qb[:], cos[:, tsl])
                            nc.gpsimd.tensor_sub(qkcat[0:64, h0, tsl],
                                                 t1[0:64, :], t2[0:64, :])
                            nc.gpsimd.tensor_sub(qkcat[0:64, h1, tsl],
                                                 t1[64:128, :], t2[64:128, :])
                            nc.gpsimd.tensor_add(qkcat[64:128, h0, tsl],
                                                 t3[0:64, :], t4[0:64, :])
                            nc.gpsimd.tensor_add(qkcat[64:128, h1, tsl],
                                                 t3[64:128, :], t4[64:128, :])

                    # v: natural [t, c] psums + Karatsuba into vcat.
                    # Reuses the q/k psum buffers (first 256 cols) so the
                    # PSUM footprint stays at 6 banks.
                    for sub in range(4):
                        tq = 4 * tb + sub
                        v1 = pp.tile([128, 512], F32, name="m1")[:, 0:C]
                        v2 = pp.tile([128, 512], F32, name="m2")[:, 0:C]
                        v3 = pp.tile([128, 512], F32, name="m3")[:, 0:C]
                        xsl = ts(sub, 128)
                        for dc in range(DC):
                            nc.tensor.matmul(
                                v1, lhsT=xr[:, dc, xsl],
                                rhs=wvs["r"][:, dc, :],
                                start=(dc == 0), stop=(dc == DC - 1))
                        for dc in range(DC):
                            nc.tensor.matmul(
                                v2, lhsT=xi[:, dc, xsl],
                                rhs=wvs["i"][:, dc, :],
                                start=(dc == 0), stop=(dc == DC - 1))
                        for dc in range(DC):
                            nc.tensor.matmul(
                                v3, lhsT=xs[:, dc, xsl],
                                rhs=wvs["s"][:, dc, :],
                                start=(dc == 0), stop=(dc == DC - 1))
                        # all three combines read PSUM -> DVE only
                        m1h = v1.rearrange("p (h d) -> p h d", h=HC)
                        m3h = v3.rearrange("p (h d) -> p h d", h=HC)
                        VS = rt.tile([128, HC, 64], F32, name="VS")
                        v2s = rt.tile([128, HC, 64], F32, name="v2s")
                        nc.scalar.copy(
                            v2s[:], v2.rearrange("p (h d) -> p h d", h=HC))
                        nc.vector.tensor_sub(vcat[:, tq, :, 0:64],
                                             m1h, v2s[:])
                        nc.vector.tensor_add(VS[:], m1h, v2s[:])
                        nc.vector.tensor_sub(vcat[:, tq, :, 64:128],
                                             m3h, VS[:])

            # ---------------- attention phase ----------------
            with tc.tile_pool(name="att", bufs=4) as att, \
                 tc.tile_pool(name="attsm", bufs=2) as attsm, \
                 tc.tile_pool(name="sp", bufs=2, space="PSUM") as sp, \
                 tc.tile_pool(name="avp", bufs=1, space="PSUM") as avp, \
                 tc.tile_pool(name="dp", bufs=1, space="PSUM") as dp:
                for h in range(HC):
                    ucc, up0 = h // 2, (h % 2) * 64
                    for iw in range(TW):
                        isl = ts(iw, 1024)
                        av = avp.tile([128, 1024], F32, name="av")
                        dn = dp.tile([1, 1024], F32, name="dn")
                        # odd-jc softmax denominators go to the Pool engine
                        # (partition_all_reduce), halving the PE's dn
                        # matmul load; partials land in dnst and a strided
                        # DVE reduce folds them back in at the end.
                        dnst = attsm.tile([128, TQ // 2, 1024], F32,
                                          name="dnst")
                        es_t = [None] * TQ

                        def issue_s(jc):
                            s = sp.tile([128, 1024], F32, name="s")
                            for half in range(2):
                                nc.tensor.matmul(
                                    s[:, ts(half, 512)],
                                    lhsT=qkcat[:, HC + h, ts(jc, 128)],
                                    rhs=qkcat[:, h, ts(2 * iw + half, 512)],
                                    start=True, stop=True)
                            es = att.tile([128, 1024], BF16, name="es")
                            nc.scalar.activation(es[:], s[:], EXP, scale=0.125)
                            es_t[jc] = es

                        issue_s(0)
                        for jc in range(TQ):
                            if jc + 1 < TQ:
                                issue_s(jc + 1)
                            es = es_t[jc]
                            # av halves back-to-back, then dn: fewer
                            # LDWEIGHTS swaps per jc
                            for half in range(2):
                                nc.tensor.matmul(av[:, ts(half, 512)],
                                                 lhsT=vcat[:, jc, h, :],
                                                 rhs=es[:, ts(half, 512)],
                                                 start=(jc == 0),
                                                 stop=(jc == TQ - 1))
                            if jc % 2 == 0:
                                for half in range(2):
                                    nc.tensor.matmul(dn[:, ts(half, 512)],
                                                     lhsT=ones[:],
                                                     rhs=es[:, ts(half, 512)],
                                                     start=(jc == 0),
                                                     stop=(jc == TQ - 2))
                            else:
                                nc.gpsimd.partition_all_reduce(
                                    dnst[:, jc // 2, :], es[:],
                                    channels=128,
                                    reduce_op=bass_isa.ReduceOp.add)
                        # free the av bank fast (DVE copy), then normalize
                        avsb = attsm.tile([128, 1024], F32, name="avsb")
                        nc.vector.tensor_scalar_mul(avsb[:], av[:], 1.0)
                        dng = attsm.tile([1, 1024], F32, name="dng")
                        nc.vector.tensor_reduce(
                            dng[:],
                            dnst[0:1, :, :].rearrange("p g q -> p q g"),
                            axis=mybir.AxisListType.X,
                            op=mybir.AluOpType.add)
                        dnf = attsm.tile([1, 1024], F32, name="dnf")
                        nc.vector.tensor_add(dnf[:], dng[:], dn[:])
                        rec = attsm.tile([1, 1024], F32, name="rec")
                        nc.vector.reciprocal_approx_fast(rec[:], dnf[:])
                        bc = attsm.tile([128, 1024], F32, name="bc")
                        nc.gpsimd.partition_broadcast(bc[:], rec[:])
                        nc.vector.tensor_mul(urt[up0:up0 + 64, ucc, isl],
                                             avsb[0:64, :], bc[0:64, :])
                        nc.vector.tensor_mul(uit[up0:up0 + 64, ucc, isl],
                                             avsb[64:128, :], bc[64:128, :])

            # ---------------- output projection ----------------
            with tc.tile_pool(name="ost", bufs=3) as ost, \
                 tc.tile_pool(name="op", bufs=2, space="PSUM") as op:
                for tq in range(TQ):
                    tslq = ts(tq, 128)
                    por = op.tile([128, 1024], F32, name="opa")
                    poi = op.tile([128, 1024], F32, name="opb")
                    for oc in range(2):
                        osl = ts(oc, 512)
                        nc.tensor.matmul(por[:, osl], lhsT=urt[:, 0, tslq],
                                         rhs=ows["r"][:, 0, osl],
                                         start=True, stop=False)
                        nc.tensor.matmul(por[:, osl], lhsT=urt[:, 1, tslq],
                                         rhs=ows["r"][:, 1, osl],
                                         start=False, stop=False)
                        nc.tensor.matmul(por[:, osl], lhsT=uit[:, 0, tslq],
                                         rhs=ows["n"][:, 0, osl],
                                         start=False, stop=False)
                        nc.tensor.matmul(por[:, osl], lhsT=uit[:, 1, tslq],
                                         rhs=ows["n"][:, 1, osl],
                                         start=False, stop=True)
                        nc.tensor.matmul(poi[:, osl], lhsT=urt[:, 0, tslq],
                                         rhs=ows["i"][:, 0, osl],
                                         start=True, stop=False)
                        nc.tensor.matmul(poi[:, osl], lhsT=urt[:, 1, tslq],
                                         rhs=ows["i"][:, 1, osl],
                                         start=False, stop=False)
                        nc.tensor.matmul(poi[:, osl], lhsT=uit[:, 0, tslq],
                                         rhs=ows["r"][:, 0, osl],
                                         start=False, stop=False)
                        nc.tensor.matmul(poi[:, osl], lhsT=uit[:, 1, tslq],
                                         rhs=ows["r"][:, 1, osl],
                                         start=False, stop=True)
                    st = ost.tile([128, 1024], F32, name="st")
                    nc.scalar.copy(st[:], por[:])
                    nc.sync.dma_start(outr_d[tslq, :], st[:])
                    sti = ost.tile([128, 1024], F32, name="sti")
                    nc.scalar.copy(sti[:], poi[:])
                    nc.sync.dma_start(outi_d[tslq, :], sti[:])

    nc.compile()
    return nc


def _to_bf16_kxm(arr, parts=128):
    """[K, M] fp32 -> [128, K//128, M] bf16 with K split as (chunk, part)."""
    k, m = arr.shape
    out = arr.reshape(k // parts, parts, m).transpose(1, 0, 2)
    return np.ascontiguousarray(out.astype(ml_dtypes.bfloat16))


def _rope_tables():
    inv_freq = 1.0 / (10000.0 ** (np.arange(0, HD, 2, dtype=np.float64) / HD))
    invf64 = np.concatenate([inv_freq, inv_freq])          # [64]
    ang = invf64[:, None] * np.arange(T, dtype=np.float64)[None, :]  # [64, T]
    cos2 = np.tile(np.cos(ang), (2, 1)).astype(np.float32)
    sin2 = np.tile(np.sin(ang), (2, 1)).astype(np.float32)
    return np.ascontiguousarray(cos2), np.ascontiguousarray(sin2)


def kernel(x_real, x_imag, q_wr, q_wi, k_wr, k_wi, v_wr, v_wi, o_wr, o_wi):
    global _COMPILED, LAST_RESULTS
    if _COMPILED is None:
        _COMPILED = _build()
    nc = _COMPILED

    cos2, sin2 = _rope_tables()
    cos2 = cos2.astype(ml_dtypes.bfloat16)
    sin2 = sin2.astype(ml_dtypes.bfloat16)
    xt = {}
    for b in range(B):
        xt[("r", b)] = _to_bf16_kxm(np.asarray(x_real[b]).T.astype(np.float32))
        xt[("i", b)] = _to_bf16_kxm(np.asarray(x_imag[b]).T.astype(np.float32))

    in_maps = []
    for core in range(NCORE):
        b, g = core // TP, core % TP
        cols = slice(g * C, (g + 1) * C)
        m = {"xrT": xt[("r", b)], "xiT": xt[("i", b)],
             "cos2": cos2, "sin2": sin2}
        for nm, wr_, wi_ in (("wq", q_wr, q_wi), ("wk", k_wr, k_wi),
                             ("wv", v_wr, v_wi)):
            wrc = np.asarray(wr_[:, cols])
            wic = np.asarray(wi_[:, cols])
            m[f"{nm}_r"] = _to_bf16_kxm(wrc)
            m[f"{nm}_i"] = _to_bf16_kxm(wic)
            m[f"{nm}_s"] = _to_bf16_kxm(wrc + wic)
        m["ow_r"] = _to_bf16_kxm(np.asarray(o_wr[cols, :]))
        m["ow_i"] = _to_bf16_kxm(np.asarray(o_wi[cols, :]))
        m["ow_n"] = _to_bf16_kxm(-np.asarray(o_wi[cols, :]))
        in_maps.append(m)

    res = run_bass_kernel_spmd(nc, in_maps, core_ids=list(range(NCORE)))
    LAST_RESULTS = res

    final_r = np.zeros((B, T, D), np.float32)
    final_i = np.zeros((B, T, D), np.float32)
    for core in range(NCORE):
        b = core // TP
        final_r[b] += res.results[core]["out_r"]
        final_i[b] += res.results[core]["out_i"]
    return final_r, final_i


# revision 30
# speedup vs baseline: 1.7028x; 1.7028x over previous
"""ComplexAttentionV3 Trainium2 kernel (v3).

Sharding: 8 cores = data-parallel over batch (2) x tensor-parallel over
heads (16 -> 4 per core). Each core computes q/k/v for its 4 heads
(column-sharded projections), local attention, and a row-sharded
o-projection producing a partial [T, D] output; the host sums the 4
partials per batch.

v3 notes vs v2:
- Karatsuba complex projections for q/k/v: m1 = xr@wr, m2 = xi@wi,
  m3 = (xr+xi)@(wr+wi); qr = m1-m2, qi = m3-m1-m2. 3 contraction
  passes instead of 4 (-25% PE work). xs and ws are precomputed on the
  host. Combines + RoPE run on DVE and Pool (gpsimd) in alternating
  blocks to balance engine load.
- x streamed t-major (512-col superblocks, double-buffered) so three
  x images fit in SBUF alongside everything else.
- Attention jc-loop software-pipelined: s(jc+1) issues before
  av/dn(jc) so the PE keeps working while the scalar engine computes
  exp. PSUM: 2 score bufs (4 banks) + av (2) + dn (2) = 8.
- av PSUM bank freed via a fast Pool-engine copy before the (slower)
  normalize chain.
- ow weights DMA'd at kernel start (no attn->oproj DMA bubble); wq
  DMA'd per-dc so the first matmul starts within ~1 MB of traffic.
"""

import numpy as np
import ml_dtypes

import concourse.bacc as bacc
import concourse.tile as tile
from concourse import bass_isa, mybir
from concourse.bass import ts
from concourse.bass_utils import run_bass_kernel_spmd

B, T, D, H = 2, 2048, 1024, 16
HD = 64
NCORE = 8
TP = 4               # head-parallel degree (per batch)
HC = H // TP         # heads per core = 4
C = HC * HD          # local channels = 256
DC = D // 128        # contraction chunks = 8
TQ = T // 128        # 128-row t-chunks = 16
TB = T // 512        # 512-col t-superblocks = 4
TW = T // 1024       # 1024-col attention q-blocks = 2

F32 = mybir.dt.float32
BF16 = mybir.dt.bfloat16
EXP = mybir.ActivationFunctionType.Exp

LAST_RESULTS = None
_COMPILED = None


def _build():
    nc = bacc.Bacc("TRN2", target_bir_lowering=False, debug=False,
                   num_devices=NCORE)

    def din(name, shape, dt=BF16):
        return nc.dram_tensor(name, shape, dt, kind="ExternalInput").ap()

    xr_d = din("xrT", [128, DC, T])
    xi_d = din("xiT", [128, DC, T])
    wq = {k: din(f"wq_{k}", [128, DC, C]) for k in ("r", "i", "s")}
    wk = {k: din(f"wk_{k}", [128, DC, C]) for k in ("r", "i", "s")}
    wv = {k: din(f"wv_{k}", [128, DC, C]) for k in ("r", "i", "s")}
    ow = {k: din(f"ow_{k}", [128, 2, D]) for k in ("r", "i", "n")}
    cos_d = din("cos2", [128, T], BF16)
    sin_d = din("sin2", [128, T], BF16)
    outr_d = nc.dram_tensor("out_r", [T, D], F32, kind="ExternalOutput").ap()
    outi_d = nc.dram_tensor("out_i", [T, D], F32, kind="ExternalOutput").ap()

    with tile.TileContext(nc) as tc:
        with tc.tile_pool(name="persist", bufs=1) as persist:
            qkcat = persist.tile([128, 2 * HC, T], BF16, name="qkcat")
            vcat = persist.tile([128, TQ, HC, 128], BF16, name="vcat")
            urt = persist.tile([128, 2, T], BF16, name="urt")
            uit = persist.tile([128, 2, T], BF16, name="uit")
            ones = persist.tile([128, 1], BF16, name="ones")
            ows = {k: persist.tile([128, 2, D], BF16, name=f"ow{k}")
                   for k in ("r", "i", "n")}
            nc.vector.memset(ones[:], 1.0)

            # ---------------- projection phase ----------------
            with tc.tile_pool(name="xw", bufs=1) as xw, \
                 tc.tile_pool(name="xst", bufs=2) as xst, \
                 tc.tile_pool(name="rt", bufs=3) as rt, \
                 tc.tile_pool(name="pp", bufs=2, space="PSUM") as pp:
                wqs = {k: xw.tile([128, DC, C], BF16, name=f"wq{k}")
                       for k in ("r", "i", "s")}
                wks = {k: xw.tile([128, DC, C], BF16, name=f"wk{k}")
                       for k in ("r", "i", "s")}
                wvs = {k: xw.tile([128, DC, C], BF16, name=f"wv{k}")
                       for k in ("r", "i", "s")}
                cos = xw.tile([128, T], BF16, name="cos")
                sin = xw.tile([128, T], BF16, name="sin")
                # DMA order = need order. wq per-dc so the PE starts after
                # ~200 KB; tables arrive before the first combine; wk/wv/ow
                # whole-tensor behind them. x rides the sync queue.
                for k in ("r", "i", "s"):
                    for dc in range(DC):
                        nc.scalar.dma_start(wqs[k][:, dc, :], wq[k][:, dc, :])
                nc.scalar.dma_start(cos[:], cos_d[:])
                nc.scalar.dma_start(sin[:], sin_d[:])
                for k in ("r", "i", "s"):
                    nc.scalar.dma_start(wks[k][:], wk[k][:])

                for tb in range(TB):
                    tsl = ts(tb, 512)
                    # wv arrives just before tb0's v-section; ow (needed
                    # only at the o-projection) queues after tb0 so neither
                    # steals DMA bandwidth from the x/wq/wk startup window.
                    if tb == 0:
                        for k in ("r", "i", "s"):
                            nc.scalar.dma_start(wvs[k][:], wv[k][:])
                    if tb == 1:
                        for k in ("r", "i", "n"):
                            nc.scalar.dma_start(ows[k][:], ow[k][:])
                    xr = xst.tile([128, DC, 512], BF16, name="xr")
                    xi = xst.tile([128, DC, 512], BF16, name="xi")
                    xs = xst.tile([128, DC, 512], BF16, name="xs")
                    for dc in range(DC):
                        nc.sync.dma_start(xr[:, dc, :], xr_d[:, dc, tsl])
                        nc.sync.dma_start(xi[:, dc, :], xi_d[:, dc, tsl])
                    # xs = xr + xi on-device (bf16 2x DVE) instead of a third
                    # 4 MB DMA stream
                    nc.vector.tensor_add(xs[:], xr[:], xi[:])

                    # q/k: transposed [c, t] psums + Karatsuba + RoPE
                    for wsrc, hbase in ((wqs, 0), (wks, HC)):
                        for cc in range(2):
                            h0, h1 = hbase + 2 * cc, hbase + 2 * cc + 1
                            csl = ts(cc, 128)
                            m1 = pp.tile([128, 512], F32, name="m1")
                            m2 = pp.tile([128, 512], F32, name="m2")
                            m3 = pp.tile([128, 512], F32, name="m3")
                            for dc in range(DC):
                                nc.tensor.matmul(
                                    m1[:], lhsT=wsrc["r"][:, dc, csl],
                                    rhs=xr[:, dc, :],
                                    start=(dc == 0), stop=(dc == DC - 1))
                            for dc in range(DC):
                                nc.tensor.matmul(
                                    m2[:], lhsT=wsrc["i"][:, dc, csl],
                                    rhs=xi[:, dc, :],
                                    start=(dc == 0), stop=(dc == DC - 1))
                            for dc in range(DC):
                                nc.tensor.matmul(
                                    m3[:], lhsT=wsrc["s"][:, dc, csl],
                                    rhs=xs[:, dc, :],
                                    start=(dc == 0), stop=(dc == DC - 1))
                            # combine + rope, alternating DVE / Pool
                            # combine + rope (P = qr, Q = qi pre-rope).
                            # GPSIMD can't touch PSUM and DVE ops may read
                            # at most one PSUM operand, so: scalar engine
                            # stages m2 into SBUF; DVE does the 3
                            # PSUM-reading combines (P/Q written as bf16)
                            # and the 4 rope muls (all-bf16 -> 2x rate);
                            # Pool does the 4 SBUF-only half add/subs.
                            m2s = rt.tile([128, 512], F32, name="m2s")
                            nc.scalar.copy(m2s[:], m2[:])
                            Pb = rt.tile([128, 512], BF16, name="Pb")
                            Sx = rt.tile([128, 512], F32, name="Sx")
                            Qb = rt.tile([128, 512], BF16, name="Qb")
                            nc.vector.tensor_sub(Pb[:], m1[:], m2s[:])
                            nc.vector.tensor_add(Sx[:], m1[:], m2s[:])
                            nc.vector.tensor_sub(Qb[:], m3[:], Sx[:])
                            t1 = rt.tile([128, 512], BF16, name="t1")
                            t2 = rt.tile([128, 512], BF16, name="t2")
                            t3 = rt.tile([128, 512], BF16, name="t3")
                            t4 = rt.tile([128, 512], BF16, name="t4")
                            nc.vector.tensor_mul(t1[:], Pb[:], cos[:, tsl])
                            nc.vector.tensor_mul(t2[:], Qb[:], sin[:, tsl])
                            nc.vector.tensor_mul(t3[:], Pb[:], sin[:, tsl])
                            nc.vector.tensor_mul(t4[:], Qb[:], cos[:, tsl])
                            nc.gpsimd.tensor_sub(qkcat[0:64, h0, tsl],
                                                 t1[0:64, :], t2[0:64, :])
                            nc.gpsimd.tensor_sub(qkcat[0:64, h1, tsl],
                                                 t1[64:128, :], t2[64:128, :])
                            nc.gpsimd.tensor_add(qkcat[64:128, h0, tsl],
                                                 t3[0:64, :], t4[0:64, :])
                            nc.gpsimd.tensor_add(qkcat[64:128, h1, tsl],
                                                 t3[64:128, :], t4[64:128, :])

                    # v: natural [t, c] psums + Karatsuba into vcat.
                    # Reuses the q/k psum buffers (first 256 cols) so the
                    # PSUM footprint stays at 6 banks.
                    for sub in range(4):
                        tq = 4 * tb + sub
                        v1 = pp.tile([128, 512], F32, name="m1")[:, 0:C]
                        v2 = pp.tile([128, 512], F32, name="m2")[:, 0:C]
                        v3 = pp.tile([128, 512], F32, name="m3")[:, 0:C]
                        xsl = ts(sub, 128)
                        for dc in range(DC):
                            nc.tensor.matmul(
                                v1, lhsT=xr[:, dc, xsl],
                                rhs=wvs["r"][:, dc, :],
                                start=(dc == 0), stop=(dc == DC - 1))
                        for dc in range(DC):
                            nc.tensor.matmul(
                                v2, lhsT=xi[:, dc, xsl],
                                rhs=wvs["i"][:, dc, :],
                                start=(dc == 0), stop=(dc == DC - 1))
                        for dc in range(DC):
                            nc.tensor.matmul(
                                v3, lhsT=xs[:, dc, xsl],
                                rhs=wvs["s"][:, dc, :],
                                start=(dc == 0), stop=(dc == DC - 1))
                        # all three combines read PSUM -> DVE only
                        m1h = v1.rearrange("p (h d) -> p h d", h=HC)
                        m3h = v3.rearrange("p (h d) -> p h d", h=HC)
                        VS = rt.tile([128, HC, 64], F32, name="VS")
                        v2s = rt.tile([128, HC, 64], F32, name="v2s")
                        nc.scalar.copy(
                            v2s[:], v2.rearrange("p (h d) -> p h d", h=HC))
                        nc.vector.tensor_sub(vcat[:, tq, :, 0:64],
                                             m1h, v2s[:])
                        nc.vector.tensor_add(VS[:], m1h, v2s[:])
                        nc.vector.tensor_sub(vcat[:, tq, :, 64:128],
                                             m3h, VS[:])

            # ---------------- attention phase ----------------
            with tc.tile_pool(name="att", bufs=4) as att, \
                 tc.tile_pool(name="attsm", bufs=2) as attsm, \
                 tc.tile_pool(name="sp", bufs=2, space="PSUM") as sp, \
                 tc.tile_pool(name="avp", bufs=1, space="PSUM") as avp, \
                 tc.tile_pool(name="dp", bufs=1, space="PSUM") as dp:
                for h in range(HC):
                    ucc, up0 = h // 2, (h % 2) * 64
                    for iw in range(TW):
                        isl = ts(iw, 1024)
                        av = avp.tile([128, 1024], F32, name="av")
                        dn = dp.tile([1, 1024], F32, name="dn")
                        es_t = [None] * TQ

                        def issue_s(jc):
                            s = sp.tile([128, 1024], F32, name="s")
                            for half in range(2):
                                nc.tensor.matmul(
                                    s[:, ts(half, 512)],
                                    lhsT=qkcat[:, HC + h, ts(jc, 128)],
                                    rhs=qkcat[:, h, ts(2 * iw + half, 512)],
                                    start=True, stop=True)
                            es = att.tile([128, 1024], BF16, name="es")
                            nc.scalar.activation(es[:], s[:], EXP, scale=0.125)
                            es_t[jc] = es

                        issue_s(0)
                        for jc in range(TQ):
                            if jc + 1 < TQ:
                                issue_s(jc + 1)
                            es = es_t[jc]
                            # av halves back-to-back, then dn: fewer
                            # LDWEIGHTS swaps per jc
                            for half in range(2):
                                nc.tensor.matmul(av[:, ts(half, 512)],
                                                 lhsT=vcat[:, jc, h, :],
                                                 rhs=es[:, ts(half, 512)],
                                                 start=(jc == 0),
                                                 stop=(jc == TQ - 1))
                            for half in range(2):
                                nc.tensor.matmul(dn[:, ts(half, 512)],
                                                 lhsT=ones[:],
                                                 rhs=es[:, ts(half, 512)],
                                                 start=(jc == 0),
                                                 stop=(jc == TQ - 1))
                        # free the av bank fast (DVE copy), then normalize
                        avsb = attsm.tile([128, 1024], F32, name="avsb")
                        nc.vector.tensor_scalar_mul(avsb[:], av[:], 1.0)
                        rec = attsm.tile([1, 1024], F32, name="rec")
                        nc.vector.reciprocal_approx_fast(rec[:], dn[:])
                        bc = attsm.tile([128, 1024], F32, name="bc")
                        nc.gpsimd.partition_broadcast(bc[:], rec[:])
                        nc.vector.tensor_mul(urt[up0:up0 + 64, ucc, isl],
                                             avsb[0:64, :], bc[0:64, :])
                        nc.vector.tensor_mul(uit[up0:up0 + 64, ucc, isl],
                                             avsb[64:128, :], bc[64:128, :])

            # ---------------- output projection ----------------
            with tc.tile_pool(name="ost", bufs=3) as ost, \
                 tc.tile_pool(name="op", bufs=2, space="PSUM") as op:
                for tq in range(TQ):
                    tslq = ts(tq, 128)
                    por = op.tile([128, 1024], F32, name="opa")
                    poi = op.tile([128, 1024], F32, name="opb")
                    for oc in range(2):
                        osl = ts(oc, 512)
                        nc.tensor.matmul(por[:, osl], lhsT=urt[:, 0, tslq],
                                         rhs=ows["r"][:, 0, osl],
                                         start=True, stop=False)
                        nc.tensor.matmul(por[:, osl], lhsT=urt[:, 1, tslq],
                                         rhs=ows["r"][:, 1, osl],
                                         start=False, stop=False)
                        nc.tensor.matmul(por[:, osl], lhsT=uit[:, 0, tslq],
                                         rhs=ows["n"][:, 0, osl],
                                         start=False, stop=False)
                        nc.tensor.matmul(por[:, osl], lhsT=uit[:, 1, tslq],
                                         rhs=ows["n"][:, 1, osl],
                                         start=False, stop=True)
                        nc.tensor.matmul(poi[:, osl], lhsT=urt[:, 0, tslq],
                                         rhs=ows["i"][:, 0, osl],
                                         start=True, stop=False)
                        nc.tensor.matmul(poi[:, osl], lhsT=urt[:, 1, tslq],
                                         rhs=ows["i"][:, 1, osl],
                                         start=False, stop=False)
                        nc.tensor.matmul(poi[:, osl], lhsT=uit[:, 0, tslq],
                                         rhs=ows["r"][:, 0, osl],
                                         start=False, stop=False)
                        nc.tensor.matmul(poi[:, osl], lhsT=uit[:, 1, tslq],
                                         rhs=ows["r"][:, 1, osl],
                                         start=False, stop=True)
                    st = ost.tile([128, 1024], F32, name="st")
                    nc.scalar.copy(st[:], por[:])
                    nc.sync.dma_start(outr_d[tslq, :], st[:])
                    sti = ost.tile([128, 1024], F32, name="sti")
                    nc.scalar.copy(sti[:], poi[:])
                    nc.sync.dma_start(outi_d[tslq, :], sti[:])

    nc.compile()
    return nc


def _to_bf16_kxm(arr, parts=128):
    """[K, M] fp32 -> [128, K//128, M] bf16 with K split as (chunk, part)."""
    k, m = arr.shape
    out = arr.reshape(k // parts, parts, m).transpose(1, 0, 2)
    return np.ascontiguousarray(out.astype(ml_dtypes.bfloat16))


def _rope_tables():
    inv_freq = 1.0 / (10000.0 ** (np.arange(0, HD, 2, dtype=np.float64) / HD))
    invf64 = np.concatenate([inv_freq, inv_freq])          # [64]
    ang = invf64[:, None] * np.arange(T, dtype=np.float64)[None, :]  # [64, T]
    cos2 = np.tile(np.cos(ang), (2, 1)).astype(np.float32)
    sin2 = np.tile(np.sin(ang), (2, 1)).astype(np.float32)
    return np.ascontiguousarray(cos2), np.ascontiguousarray(sin2)


def kernel(x_real, x_imag, q_wr, q_wi, k_wr, k_wi, v_wr, v_wi, o_wr, o_wi):
    global _COMPILED, LAST_RESULTS
    if _COMPILED is None:
        _COMPILED = _build()
    nc = _COMPILED

    cos2, sin2 = _rope_tables()
    cos2 = cos2.astype(ml_dtypes.bfloat16)
    sin2 = sin2.astype(ml_dtypes.bfloat16)
    xt = {}
    for b in range(B):
        xt[("r", b)] = _to_bf16_kxm(np.asarray(x_real[b]).T.astype(np.float32))
        xt[("i", b)] = _to_bf16_kxm(np.asarray(x_imag[b]).T.astype(np.float32))

    in_maps = []
    for core in range(NCORE):
        b, g = core // TP, core % TP
        cols = slice(g * C, (g + 1) * C)
        m = {"xrT": xt[("r", b)], "xiT": xt[("i", b)],
             "cos2": cos2, "sin2": sin2}
        for nm, wr_, wi_ in (("wq", q_wr, q_wi), ("wk", k_wr, k_wi),
                             ("wv", v_wr, v_wi)):
            wrc = np.asarray(wr_[:, cols])
            wic = np.asarray(wi_[:, cols])
            m[f"{nm}_r"] = _to_bf16_kxm(wrc)
            m[f"{nm}_i"] = _to_bf16_kxm(wic)
            m[f"{nm}_s"] = _to_bf16_kxm(wrc + wic)
        m["ow_r"] = _to_bf16_kxm(np.asarray(o_wr[cols, :]))
        m["ow_i"] = _to_bf16_kxm(np.asarray(o_wi[cols, :]))
        m["ow_n"] = _to_bf16_kxm(-np.asarray(o_wi[cols, :]))
        in_maps.append(m)

    res = run_bass_kernel_spmd(nc, in_maps, core_ids=list(range(NCORE)))
    LAST_RESULTS = res

    final_r = np.zeros((B, T, D), np.float32)
    final_i = np.zeros((B, T, D), np.float32)
    for core in range(NCORE):
        b = core // TP
        final_r[b] += res.results[core]["out_r"]
        final_i[b] += res.results[core]["out_i"]
    return final_r, final_i
